# revision 37
# baseline (speedup 1.0000x reference)
"""Multi-head causal attention with RoPE on 8 TRN2 NeuronCores.

Problem: B=2, T=2048, D=1024, H=16 heads (dh=64), fp32 I/O.
  q/k/v = x @ w{q,k,v}.T ; RoPE(q,k) ; causal softmax((q k^T)/sqrt(dh)) @ v ;
  out = concat_heads @ wo.T

Sharding (8 cores): head-parallel compute, token-striped output. Core c owns
heads {2c, 2c+1} for both batches; four AllToAll collectives redistribute
attention outputs so core c ends up with all 1024 features for its four
128-token chunks {c, c+8, c+16, c+24}; it then applies the full output
projection for those chunks. The host interleaves the chunks back.

Scheduling notes (hard-won, via per-instruction NTFF traces):
 - All heavy inputs are host-packed into the exact SBUF layout so each DMA is
   one descriptor with 2KB+ contiguous bytes per partition.
 - PV trails QK by two chunks (pt bufs=4) so the scalar-engine exp latency
   (~1.1us) plus semaphore hops never block the in-order PE queue.
 - A dummy 8-rank AllToAll is dispatched first thing: the collective
   subsystem's bootstrap (~40-100us, serial on the CC queue) runs concurrently
   with the projection phase instead of delaying a2a(0).
 - A collective_compute dispatch BLOCKS the issuing (Pool) queue until the CC
   core accepts it (CC init ~20us; first-collective bootstrap 38..126us,
   variable). Pool is therefore DISPATCH-ONLY: the causal-mask muls AND the
   rope sin-muls both run on DVE. With masks on Pool, every dispatch sat
   behind the tile's exp-gated mask chain (cc1..cc3 fired 40-50us late);
   with rope-sins on Pool, a bootstrap-blocked dispatch froze the next
   tiles' rope -> QK -> the whole attention stream for ~70us.
 - The v1 ones-row memset runs BEFORE the dummy dispatch (CC-core init
   blocks Pool ~20us at boot; the first PV needs that row).
 - Each tile's epilogue part-B (denominator broadcast via a K=2 sel-matmul,
   onr muls, a2a staging, collective dispatch) is deferred INTO the next
   attention tile (invoked after its chunk 2): left between tiles, the whole
   DVE reciprocal chain sat on the in-order PE queue's critical path
   (10-15us per tile boundary in the trace).
 - ALL a2a_out -> SBUF loads and ALL final projections run in a tail after
   the last attention tile, on the sync+scalar queues only. A DMA waiting on
   a collective semaphore blocks its whole queue, so mid-schedule at-loads
   froze the pipeline whenever a collective ran late (71us once); and a
   final-projection matmul emitted before its at-data landed stalls the
   in-order PE queue the same way. The tail costs ~10us vs the fully
   overlapped ideal but is robust to bootstrap/collective latency variance
   (observed 38us..160us for the same code).
 - Epilogue staging DMAs must stay on the SYNC queue: staging from the
   scalar queue intermittently raced the collective's packet-count wait
   (nondeterministic rel_err 1e-1 in one of two runs).
 - NO gpsimd custom-ISA ops: each Pool microcode library swap costs ~6us of
   dead time.
 - Engine load map: Scalar = exps + vtt copies + prefetch DMAs; DVE = rope
   shuffle/cos-mul/sin-mul/add, causal mask muls, o65+fo psum copies,
   v-transpose copies, onr muls, fast reciprocal; Pool = memsets + collective
   dispatches only.
"""

import numpy as np
import ml_dtypes

import concourse.bacc as bacc
import concourse.tile as tile
import concourse.mybir as mybir
from concourse import bass_utils

BF16 = mybir.dt.bfloat16
F32 = mybir.dt.float32
AF = mybir.ActivationFunctionType

NCORES = 8
B, T, D, H = 2, 2048, 1024, 16
DH = D // H          # 64
HPC = H // NCORES    # 2 heads per core
FPC = DH * HPC       # 128 features per core
TOK = B * T          # 4096
TPC = TOK // NCORES  # 512 tokens per core (output shard)
KC = D // 128        # 8 contraction chunks
NT = T // 512        # 4 query tiles of 512 per batch
VG = 256             # cols per v-group: [v_h0(64) | 1 | pad | v_h1(64) | 1 | pad]

_COMPILED = None


def _build():
    nc = bacc.Bacc("TRN2", target_bir_lowering=False, debug=False, num_devices=NCORES)

    xp_d = nc.dram_tensor("xp", [128, KC * TOK], BF16, kind="ExternalInput")
    wq_d = nc.dram_tensor("wqp", [128, KC * FPC], BF16, kind="ExternalInput")
    wk_d = nc.dram_tensor("wkp", [128, KC * FPC], BF16, kind="ExternalInput")
    wv_d = nc.dram_tensor("wvp", [128, KC * FPC], BF16, kind="ExternalInput")
    wo_d = nc.dram_tensor("wop", [128, KC * D], BF16, kind="ExternalInput")
    C_d = nc.dram_tensor("cosC", [128, T], BF16, kind="ExternalInput")
    S_d = nc.dram_tensor("sinS", [128, T], BF16, kind="ExternalInput")
    mask_d = nc.dram_tensor("mask", [128, 128], BF16, kind="ExternalInput")
    id_d = nc.dram_tensor("ident", [128, 128], BF16, kind="ExternalInput")
    sel_d = nc.dram_tensor("sel2", [2, 128], BF16, kind="ExternalInput")
    out_d = nc.dram_tensor("out", [TPC, D], F32, kind="ExternalOutput")

    swap16 = list(range(16, 32)) + list(range(16))

    with tile.TileContext(nc) as tc:
        with (
            tc.tile_pool(name="sb", bufs=1) as sb,
            tc.tile_pool(name="ps", bufs=1, space="PSUM") as ps,
            tc.tile_pool(name="dram", bufs=1, space="DRAM") as dram,
        ):
            # ---- persistent intermediates; the ones-row memset must precede
            # the dummy dispatch (which blocks Pool through CC-core init) ----
            qrot_sb = sb.tile([128, TOK], BF16)
            krot_sb = sb.tile([128, TOK], BF16)
            v1_sb = sb.tile([128, B * (T // 128) * VG], BF16)
            nc.gpsimd.memset(
                v1_sb[:].rearrange("p (g c) -> p g c", c=128)[:, :, 64:65], 1.0
            )

            # ---- dummy collective first: pays the CC bootstrap cost during
            # the projection phase ----
            dum_in = dram.tile([8, 16], BF16, name="dumin")
            dum_out = dram.tile([8, 16], BF16, name="dumout")
            zz = sb.tile([8, 16], BF16)
            nc.gpsimd.memset(zz[:], 0.0)
            nc.gpsimd.dma_start(dum_in[:], zz[:])
            nc.gpsimd.collective_compute(
                "AllToAll",
                mybir.AluOpType.bypass,
                replica_groups=[list(range(NCORES))],
                ins=[dum_in.opt()],
                outs=[dum_out.opt()],
            )

            # ---- prefetch: everything is host-packed, one flat DMA each.
            # scalar queue: weights + rope tables + odd x blocks + wo
            # sync queue:   even x blocks + mask + identity
            wq_sb = sb.tile([128, KC * FPC], BF16)
            wk_sb = sb.tile([128, KC * FPC], BF16)
            wv_sb = sb.tile([128, KC * FPC], BF16)
            C_sb = sb.tile([128, T], BF16)
            S_sb = sb.tile([128, T], BF16)
            mask2_sb = sb.tile([128, 256], BF16)
            id_sb = sb.tile([128, 128], BF16)
            xp_sb = sb.tile([128, KC * TOK], BF16)
            wo_sb = sb.tile([128, KC * D], BF16)

            BLK = KC * 512  # 4096 cols per (b,n) token block

            def x_block(i):
                return (
                    xp_sb[:, BLK * i : BLK * i + BLK],
                    xp_d[:, BLK * i : BLK * i + BLK],
                )

            # first token block split across both queues so the first
            # projection starts ~3us sooner (wq first on scalar: tiny)
            # tiny first transfers on both queues: the first DMA on a cold
            # queue pays ~12us bring-up (trace: first x-chunk landed at t=13);
            # a small leading descriptor absorbs it so the big loads behind
            # complete sooner
            sel2w = sb.tile([2, 128], BF16)
            nc.sync.dma_start(sel2w[:], sel_d[:])
            nc.scalar.dma_start(mask2_sb[:, 0:128], mask_d[:])
            nc.scalar.dma_start(wq_sb[:], wq_d[:])
            d, s = x_block(0)
            # quarters: the first projection matmuls only need the leading
            # columns, so the first chunk lands (and unblocks the PE) sooner
            quart = KC * 128
            nc.sync.dma_start(d[:, 0:quart], s[:, 0:quart])
            nc.sync.dma_start(d[:, quart : 2 * quart], s[:, quart : 2 * quart])
            nc.scalar.dma_start(d[:, 2 * quart : 3 * quart], s[:, 2 * quart : 3 * quart])
            nc.scalar.dma_start(d[:, 3 * quart : BLK], s[:, 3 * quart : BLK])
            nc.scalar.dma_start(wk_sb[:], wk_d[:])
            nc.scalar.dma_start(wv_sb[:], wv_d[:])
            nc.sync.dma_start(C_sb[:], C_d[:])
            nc.sync.dma_start(S_sb[:], S_d[:])
            nc.sync.dma_start(mask2_sb[:, 128:256], mask_d[:])
            nc.sync.dma_start(id_sb[:], id_d[:])
            for i in range(1, B * NT):
                d, s = x_block(i)
                (nc.scalar if i % 2 else nc.sync).dma_start(d, s)
            nc.scalar.dma_start(wo_sb[:], wo_d[:])

            # 4 AllToAll groups: group g carries global token chunks 8g+o to rank o
            a2a_in = [dram.tile([D, 128], BF16, name=f"a2ain{g}") for g in range(4)]
            a2a_out = [dram.tile([D, 128], BF16, name=f"a2aout{g}") for g in range(4)]

            # ================= filler machinery =================
            fillers = []  # FIFO of zero-arg thunks, each ~1 PE instruction

            def pull(k):
                for _ in range(k):
                    if fillers:
                        fillers.pop(0)()

            def drain():
                while fillers:
                    fillers.pop(0)()

            def rope_tile(pp, dst_sb, b, n):
                # dst = pp*C + swap16(pp)*S, reading the projection psum directly
                swp = sb.tile([128, 512], F32, tag="swp", bufs=3, name=f"swp{b}{n}")
                nc.vector.stream_shuffle(swp[:], pp[:], swap16)
                t1 = sb.tile([128, 512], BF16, tag="t1", bufs=3, name=f"t1{b}{n}")
                nc.vector.tensor_mul(t1[:], pp[:], C_sb[:, 512 * n : 512 * n + 512])
                # DVE, not Pool: Pool must stay dispatch-only — a collective
                # dispatch blocks the Pool queue until the CC core is free
                # (up to ~126us when the bootstrap runs late), and rope-sins
                # queued behind it froze the next attention tiles.
                t2 = sb.tile([128, 512], BF16, tag="t2", bufs=3, name=f"t2{b}{n}")
                nc.vector.tensor_mul(t2[:], swp[:], S_sb[:, 512 * n : 512 * n + 512])
                nc.vector.tensor_add(
                    dst_sb[:, b * T + 512 * n : b * T + 512 * n + 512], t1[:], t2[:]
                )

            def add_proj_fillers(w_sb, b, n, kind):
                """8 fillers (one matmul each). kind: 'q'|'k'|'v'."""
                st = {}
                blk = (NT * b + n) * BLK

                def mk(kc):
                    def f():
                        if kc == 0:
                            st["pp"] = ps.tile(
                                [128, 512], F32, tag="proj", bufs=2, name=f"pp{kind}{b}{n}"
                            )
                        nc.tensor.matmul(
                            st["pp"][:],
                            w_sb[:, kc * FPC : (kc + 1) * FPC],
                            xp_sb[:, blk + 512 * kc : blk + 512 * kc + 512],
                            start=(kc == 0),
                            stop=(kc == KC - 1),
                        )
                        if kc == KC - 1:
                            if kind == "q":
                                rope_tile(st["pp"], qrot_sb, b, n)
                            elif kind == "k":
                                rope_tile(st["pp"], krot_sb, b, n)
                            else:
                                vtt = sb.tile(
                                    [128, 512], BF16, tag="vtt", bufs=2, name=f"vtt{b}{n}"
                                )
                                nc.scalar.activation(vtt[:], st["pp"][:], AF.Copy)
                                for i in range(4):
                                    fillers.append(mk_transpose(vtt, b, n, i))

                    return f

                for kc in range(KC):
                    fillers.append(mk(kc))

            def mk_transpose(vtt, b, n, i):
                def f():
                    g = VG * ((T // 128) * b + 4 * n + i)
                    tp = ps.tile([128, 128], BF16, tag="proj", bufs=2, name=f"tp{b}{n}{i}")
                    nc.tensor.matmul(
                        tp[:],
                        vtt[:, 128 * i : 128 * i + 128],
                        id_sb[:],
                        is_transpose=True,
                        start=True,
                        stop=True,
                    )
                    # one strided DVE copy moves both heads' 64 columns
                    nc.vector.tensor_copy(
                        v1_sb[:, g : g + 256].rearrange("p (h c) -> p h c", h=2)[
                            :, :, 0:64
                        ],
                        tp[:].rearrange("p (h c) -> p h c", h=2),
                    )

                return f

            def add_tile(b, n):
                add_proj_fillers(wq_sb, b, n, "q")
                add_proj_fillers(wk_sb, b, n, "k")
                add_proj_fillers(wv_sb, b, n, "v")

            # ---- at tiles (a2a_out -> SBUF), split per k-chunk ----
            at_tiles = {}

            def load_at(g, queues):
                at = sb.tile([128, KC * 128], BF16, tag="at", bufs=4, name=f"at{g}")
                for kc in range(KC):
                    q = queues[kc % len(queues)]
                    q.dma_start(
                        at[:, 128 * kc : 128 * kc + 128],
                        a2a_out[g][128 * kc : 128 * kc + 128, :],
                    )
                at_tiles[g] = at

            def add_final_fillers(g):
                """16 fillers: output projection for token chunk group g."""
                st = {}

                def mk(nh, kc):
                    def f():
                        at = at_tiles[g]
                        if kc == 0:
                            st[nh] = ps.tile(
                                [128, 512], F32, tag="proj", bufs=2, name=f"fp{g}{nh}"
                            )
                        nc.tensor.matmul(
                            st[nh][:],
                            at[:, 128 * kc : 128 * kc + 128],
                            wo_sb[:, kc * D + 512 * nh : kc * D + 512 * nh + 512],
                            start=(kc == 0),
                            stop=(kc == KC - 1),
                        )
                        if kc == KC - 1:
                            fo = sb.tile(
                                [128, 512], F32, tag="fo", bufs=2, name=f"fo{g}{nh}"
                            )
                            nc.vector.tensor_copy(fo[:], st[nh][:])
                            nc.sync.dma_start(
                                out_d[128 * g : 128 * g + 128, 512 * nh : 512 * nh + 512],
                                fo[:],
                            )

                    return f

                for nh in range(2):
                    for kc in range(KC):
                        fillers.append(mk(nh, kc))

            # sel2: [2,128] selection matrix for the PE-side denominator
            # broadcast (row h -> output partitions 64h..64h+64)
            sel2 = sb.tile([2, 128], BF16)
            nc.sync.dma_start(sel2[:], sel_d[:])

            # ================= attention =================
            def attn_core(b, j, pb=None):
                """Both heads for (batch b, q-tile j). QK pairs run in disjoint
                PE row groups; exp on the scalar engine; causal mask mul on
                DVE; PV trails QK by TWO chunks. The PREVIOUS tile's epilogue
                part-B (pb = (closure, emit_cc)) is invoked after chunk 2, so
                its reciprocal chain (DVE, emitted before this tile's masks)
                is long done and the bps matmul never stalls the PE — leaving
                it between tiles put the whole DVE epilogue chain on the PE
                critical path (10-15us/tile in the v6 trace)."""
                ops = [
                    ps.tile([65, 512], F32, tag="opsum", bufs=2, name=f"op{b}{h}{j}")
                    for h in range(2)
                ]
                nch = 4 * j + 4

                def qk_exp(c):
                    diag = c - 4 * j
                    lo = 128 * diag if diag >= 0 else 0
                    sp = ps.tile(
                        [128, 1024], F32, tag="spsum", bufs=2, name=f"sp{b}{j}{c}"
                    )
                    spv = sp[:].rearrange("p (h t) -> p h t", h=2)
                    for h in range(2):
                        nc.tensor.matmul(
                            sp[:, 512 * h + lo : 512 * h + 512],
                            krot_sb[64 * h : 64 * h + 64, b * T + 128 * c : b * T + 128 * c + 128],
                            qrot_sb[
                                64 * h : 64 * h + 64,
                                b * T + 512 * j + lo : b * T + 512 * j + 512,
                            ],
                            start=True,
                            stop=True,
                        )
                    pt = sb.tile(
                        [128, 1024], BF16, tag="pt", bufs=6, name=f"pt{b}{j}{c}"
                    )
                    ptv = pt[:].rearrange("p (h t) -> p h t", h=2)
                    nc.scalar.activation(
                        ptv[:, :, lo:512], spv[:, :, lo:512], AF.Exp, scale=0.125
                    )
                    if diag >= 0:
                        # zero the upper triangle post-exp (DVE: Pool would
                        # couple the exp-gated mask chain with the collective
                        # dispatches; PE-side additive mask lost ~20us on the
                        # exp critical path)
                        nc.vector.tensor_mul(
                            ptv[:, :, lo : lo + 128], ptv[:, :, lo : lo + 128],
                            mask2_sb[:].rearrange("p (h t) -> p h t", h=2),
                        )
                    return pt

                def pv(c, pt):
                    diag = c - 4 * j
                    lo = 128 * diag if diag >= 0 else 0
                    g = VG * ((T // 128) * b + c)
                    for h in range(2):
                        nc.tensor.matmul(
                            ops[h][:, lo:512],
                            v1_sb[:, g + 128 * h : g + 128 * h + 65],
                            pt[:, 512 * h + lo : 512 * h + 512],
                            start=(c == 0),
                            stop=(c == nch - 1),
                        )

                pts = {}
                for c in range(nch):
                    pts[c] = qk_exp(c)
                    if c == 2 and pb is not None:
                        pb[0](pb[1])
                    if c >= 2:
                        pv(c - 2, pts.pop(c - 2))
                pv(nch - 2, pts.pop(nch - 2))
                pv(nch - 1, pts.pop(nch - 1))
                # psum -> SBUF copies on DVE release the opsum banks for the
                # next tile without loading the exp-critical scalar queue
                o65s = []
                for h in range(2):
                    o65 = sb.tile([65, 512], F32, tag="o65", bufs=4, name=f"o65{b}{h}{j}")
                    nc.vector.tensor_copy(o65[:], ops[h][:])
                    o65s.append(o65)
                return o65s

            def epilogue_a(b, j, o65s):
                """Immediate post-tile work (DVE + sync only, no PE/Pool):
                gather the two denominator rows, fast reciprocal, cast to
                bf16. Returns the deferred part-B closure."""
                sums = sb.tile([2, 512], F32, tag="sums", bufs=3, name=f"sums{b}{j}")
                for h in range(2):
                    nc.sync.dma_start(sums[h : h + 1, :], o65s[h][64:65, :])
                rec2 = sb.tile([2, 512], F32, tag="rec4", bufs=3, name=f"rec2{b}{j}")
                nc.vector.reciprocal_approx_fast(rec2[:], sums[:])
                recb2 = sb.tile([2, 512], BF16, tag="recb2", bufs=3, name=f"recb2{b}{j}")
                nc.vector.tensor_copy(recb2[:], rec2[:])

                def part_b(emit_cc):
                    # one K=2 PE matmul broadcasts both heads' reciprocals
                    # into a psum bank (rows 64h..64h+64 = head h)
                    bps = ps.tile([128, 512], F32, tag="spsum", bufs=2, name=f"bps{b}{j}")
                    nc.tensor.matmul(bps[:], sel2[:], recb2[:], start=True, stop=True)
                    m0 = 16 * b + 4 * j
                    o0, g = m0 % 8, m0 // 8
                    for h in range(2):
                        onr = sb.tile([64, 512], BF16, tag="onr", bufs=4, name=f"onr{b}{j}{h}")
                        nc.vector.tensor_mul(
                            onr[:], o65s[h][0:64, :], bps[64 * h : 64 * h + 64, :]
                        )
                        # all 4 chunks in one DMA: dst rows 128*(o0+i)+64h..+64
                        nc.sync.dma_start(
                            a2a_in[g][:]
                            .rearrange("(o r) t -> r o t", r=128)[
                                64 * h : 64 * h + 64, o0 : o0 + 4, :
                            ],
                            onr[:].rearrange("p (i t) -> p i t", i=4),
                        )
                    if emit_cc:
                        a2a_call(g)()

                return part_b

            def a2a_call(g):
                def f():
                    nc.gpsimd.collective_compute(
                        "AllToAll",
                        mybir.AluOpType.bypass,
                        replica_groups=[list(range(NCORES))],
                        ins=[a2a_in[g].opt()],
                        outs=[a2a_out[g].opt()],
                    )

                return f

            # ================= schedule =================
            # One-tile lookahead: attention starts right after tile (0,0)'s
            # projections; each next tile's fillers are pulled during the
            # current attention tile (1 after each QK / PV chunk) and the
            # remainder drains between tiles. Part-B closures ride inside the
            # NEXT attention tile (chunk 2).
            add_tile(0, 0)
            drain()
            add_tile(0, 1)
            drain()
            pb00 = epilogue_a(0, 0, attn_core(0, 0))
            add_tile(0, 2)
            drain()
            pb01 = epilogue_a(0, 1, attn_core(0, 1, pb=(pb00, False)))
            add_tile(0, 3)
            drain()
            pb02 = epilogue_a(0, 2, attn_core(0, 2, pb=(pb01, True)))   # cc0
            add_tile(1, 0)
            drain()
            pb03 = epilogue_a(0, 3, attn_core(0, 3, pb=(pb02, False)))
            add_tile(1, 1)
            drain()
            pb10 = epilogue_a(1, 0, attn_core(1, 0, pb=(pb03, True)))   # cc1
            add_tile(1, 2)
            drain()
            pb11 = epilogue_a(1, 1, attn_core(1, 1, pb=(pb10, False)))
            add_tile(1, 3)
            drain()
            pb12 = epilogue_a(1, 2, attn_core(1, 2, pb=(pb11, True)))   # cc2
            pb13 = epilogue_a(1, 3, attn_core(1, 3, pb=(pb12, False)))
            pb13(True)  # cc3
            # ---- tail: at loads + ALL final projections. The in-order PE
            # queue meets its first collective-gated instruction only after
            # every attention tile is done, so a late bootstrap degrades
            # gracefully instead of freezing the pipeline mid-attention. ----
            # groups 0-2 load on sync/scalar and overlap cc3's flight; at(3)
            # stays off the Pool queue (behind the cc3 dispatch it would
            # serialize after cc3 like any Pool op).
            load_at(0, [nc.sync])
            load_at(1, [nc.scalar])
            load_at(2, [nc.sync, nc.scalar])
            load_at(3, [nc.sync, nc.scalar, nc.gpsimd])
            for g in range(4):
                add_final_fillers(g)
            drain()

    nc.compile()
    return nc


def _get_compiled():
    global _COMPILED
    if _COMPILED is None:
        _COMPILED = _build()
    return _COMPILED


def _prep_in_maps(embedding_word, wq, wk, wv, wo):
    bf = ml_dtypes.bfloat16
    x = np.asarray(embedding_word, np.float32).reshape(TOK, D)
    xT = np.ascontiguousarray(x.T).astype(bf)  # [D, TOK]
    xp = np.ascontiguousarray(
        xT.reshape(KC, 128, B, NT, 512).transpose(1, 2, 3, 0, 4).reshape(128, KC * TOK)
    )

    woT = np.asarray(wo, np.float32).T  # [D, D]
    wop = np.ascontiguousarray(
        woT.reshape(KC, 128, D).transpose(1, 0, 2).reshape(128, KC * D)
    ).astype(bf)

    perm64 = [
        (2 * (16 * q + r) if r < 16 else 2 * (16 * q + (r - 16)) + 1)
        for q in range(2)
        for r in range(32)
    ]
    perm64 = np.asarray(perm64)

    freqs = 1.0 / (10000.0 ** (np.arange(0, DH, 2, dtype=np.float64) / DH))  # [32]
    ang = np.arange(T, dtype=np.float64)[:, None] * freqs[None, :]  # [T, 32]
    cos_t, sin_t = np.cos(ang), np.sin(ang)
    rows = np.arange(128)
    wh = rows % 64
    qd = wh // 32
    r32 = wh % 32
    dmap = 16 * qd + (r32 % 16)
    sign = np.where(r32 < 16, -1.0, 1.0)
    C = np.ascontiguousarray(cos_t[:, dmap].T).astype(bf)  # [128, T]
    S = np.ascontiguousarray((sin_t[:, dmap] * sign[None, :]).T).astype(bf)

    rr = np.arange(128)[:, None]
    cc = np.arange(128)[None, :]
    mask = np.where(cc >= rr, 1.0, 0.0).astype(bf)
    ident = np.eye(128, dtype=np.float32).astype(bf)
    sel2 = np.zeros((2, 128), np.float32)
    sel2[0, 0:64] = 1.0
    sel2[1, 64:128] = 1.0
    sel2 = sel2.astype(bf)

    wqf = np.asarray(wq, np.float32)
    wkf = np.asarray(wk, np.float32)
    wvf = np.asarray(wv, np.float32)

    def pack_w(w_c):
        # w_c: [FPC, D] -> transpose -> [D, FPC] -> [128, (k c)] SBUF layout
        wT = w_c.T
        return np.ascontiguousarray(
            wT.reshape(KC, 128, FPC).transpose(1, 0, 2).reshape(128, KC * FPC)
        ).astype(bf)

    in_maps = []
    for c in range(NCORES):
        rows_c = slice(FPC * c, FPC * c + FPC)
        wq_c = wqf[rows_c].reshape(HPC, DH, D)[:, perm64, :].reshape(FPC, D)
        wk_c = wkf[rows_c].reshape(HPC, DH, D)[:, perm64, :].reshape(FPC, D)
        wv_c = wvf[rows_c]
        in_maps.append(
            {
                "xp": xp,
                "wqp": pack_w(wq_c),
                "wkp": pack_w(wk_c),
                "wvp": pack_w(wv_c),
                "wop": wop,
                "cosC": C,
                "sinS": S,
                "mask": mask,
                "ident": ident,
                "sel2": sel2,
            }
        )
    return in_maps


def _unshard(core_outs):
    """core_outs[c] is [TPC, D] covering token chunks {c, 8+c, 16+c, 24+c}
    (row-blocks g=0..3). Interleave back to [B, T, D]."""
    a = np.stack(core_outs, axis=0)  # [8, TPC, D]
    a = a.reshape(NCORES, 4, 128, D).transpose(1, 0, 2, 3).reshape(TOK, D)
    return np.ascontiguousarray(a.reshape(B, T, D).astype(np.float32))


def kernel(embedding_word, wq, wk, wv, wo):
    nc = _get_compiled()
    in_maps = _prep_in_maps(embedding_word, wq, wk, wv, wo)
    res = bass_utils.run_bass_kernel_spmd(nc, in_maps, core_ids=list(range(NCORES)))
    return _unshard([res.results[c]["out"] for c in range(NCORES)])
